# revision 39
# baseline (speedup 1.0000x reference)
"""Trainium2 Bass kernel for nn_CCCrossLayerAttentionB (criss-cross cross-layer attention).

Self-contained: kernel(**inputs) -> np.ndarray [8, 512, 96, 96] fp32.

Sharding: data-parallel over batch (8 images -> 8 cores); BN stats via AllReduce.

Host-side restructuring (exact up to float assoc):
  - qf never materialized: q = (q_w @ conv1_w) @ low.
  - hf never materialized: every hf-consumer is a 1x1 conv, which commutes with the
    (separable, linear) bilinear 2x upsample, so the high stream is computed on the
    48x48 grid and upsampled afterward:
      ks = (k_w@conv2_w)@high; vs = (v_w@conv2_w)@high; xs = ((Wb_v@conv2_w+Wb_h)/16)@high
  - Device blends compute up16() = 16*up() in one fused op per output; the 1/16 is
    folded into host weights (q /16; xs /16; final att weights gamma/16).
  - vf2 never materialized:  k2' = (gamma k_w)@A1' + k1',  v2' = (gamma v_w)@A1' + v1',
    y = (gamma/16 Wb_v)@(A1'+A2') + up16(xs);  A' accumulated in place.
Attention per column w (and symmetrically per row h):
  e[H',h] = k'[:,H',w]^T q'[:,h,w] (flipped => softmax sums via ones-matmul, aggregation
  needs no attention transpose);  exp via ACT (+ -1e9 diag mask for the H direction);
  normalize exp tiles; aggregate with pixel-major v slices obtained from per-column
  PE transposes.
Everything stays in SBUF: A' is accumulated h-major so the final projection, BN stats
and BN apply all read/write contiguous APs; y reuses the attention tensors' SBUF and
only the fp32 output leaves the chip (large contiguous DMAs).
"""
import numpy as np
import ml_dtypes

import concourse.bass as bass
import concourse.bacc as bacc
import concourse.tile as tile
from concourse import mybir
from concourse.bass_utils import run_bass_kernel_spmd

F32 = mybir.dt.float32
BF16 = mybir.dt.bfloat16
AL = mybir.AluOpType
AF = mybir.ActivationFunctionType

N_CORES = 8
B, C, H, W = 8, 512, 96, 96
HS = 48
PIX = H * W
PIXS = HS * HS
CIN = 256
CI = 32
NTOT = float(B * PIX)
BN_EPS = 1e-5
NEG = -1e9

_CACHE = {}


def _f32(x):
    return np.ascontiguousarray(np.asarray(x, dtype=np.float32))


# ---------------------------------------------------------------------------
# blend helpers: up16 along last dim (48 -> 96) / middle dim
# ---------------------------------------------------------------------------

def _up_last(nc, eng, out, xin):
    """xin [P, n, 48] -> out [P, n, 96], out = 16 * bilinear (x4 per axis)."""
    eng.tensor_scalar_mul(out[:, :, 0:1], xin[:, :, 0:1], 4.0)
    eng.tensor_scalar_mul(out[:, :, 95:96], xin[:, :, 47:48], 4.0)
    eng.scalar_tensor_tensor(out[:, :, 2:95:2], xin[:, :, 1:48], 3.0, xin[:, :, 0:47],
                             AL.mult, AL.add)
    eng.scalar_tensor_tensor(out[:, :, 1:94:2], xin[:, :, 0:47], 3.0, xin[:, :, 1:48],
                             AL.mult, AL.add)


def _up_mid(nc, eng, out, xin):
    """xin [P, 48, n] -> out [P, 96, n]."""
    eng.tensor_scalar_mul(out[:, 0:1, :], xin[:, 0:1, :], 4.0)
    eng.tensor_scalar_mul(out[:, 95:96, :], xin[:, 47:48, :], 4.0)
    eng.scalar_tensor_tensor(out[:, 2:95:2, :], xin[:, 1:48, :], 3.0, xin[:, 0:47, :],
                             AL.mult, AL.add)
    eng.scalar_tensor_tensor(out[:, 1:94:2, :], xin[:, 0:47, :], 3.0, xin[:, 1:48, :],
                             AL.mult, AL.add)


def _up_mid_tt(nc, eng, out, xin, x3):
    """Same as _up_mid but pure tensor-tensor adds (GpSimd-safe); x3 = 3*xin."""
    eng.tensor_add(out[:, 0:1, :], x3[:, 0:1, :], xin[:, 0:1, :])
    eng.tensor_add(out[:, 95:96, :], x3[:, 47:48, :], xin[:, 47:48, :])
    eng.tensor_add(out[:, 2:95:2, :], x3[:, 1:48, :], xin[:, 0:47, :])
    eng.tensor_add(out[:, 1:94:2, :], x3[:, 0:47, :], xin[:, 1:48, :])


def _up_last_tt(nc, eng, out, xin, x3):
    """Same as _up_last but pure tensor-tensor adds (GpSimd-safe); x3 = 3*xin."""
    eng.tensor_add(out[:, :, 0:1], x3[:, :, 0:1], xin[:, :, 0:1])
    eng.tensor_add(out[:, :, 95:96], x3[:, :, 47:48], xin[:, :, 47:48])
    eng.tensor_add(out[:, :, 2:95:2], x3[:, :, 1:48], xin[:, :, 0:47])
    eng.tensor_add(out[:, :, 1:94:2], x3[:, :, 0:47], xin[:, :, 1:48])


# ---------------------------------------------------------------------------
# device kernel
# ---------------------------------------------------------------------------

def build(debug_taps=False):
    nc = bacc.Bacc("TRN2", target_bir_lowering=False, debug=False, num_devices=N_CORES)

    low_d = nc.dram_tensor("low", [C, H, W], F32, kind="ExternalInput")
    high_d = nc.dram_tensor("high", [C, HS, HS], F32, kind="ExternalInput")
    wq_d = nc.dram_tensor("wqT", [C, CI], BF16, kind="ExternalInput")
    wkvx_d = nc.dram_tensor("wkvxT", [C, CIN + C + CI], BF16, kind="ExternalInput")
    wk2_d = nc.dram_tensor("wk2T", [CIN, CI], BF16, kind="ExternalInput")
    wv2_d = nc.dram_tensor("wv2T", [CIN, CIN], BF16, kind="ExternalInput")
    wfin_d = nc.dram_tensor("wfinT", [CIN, C], BF16, kind="ExternalInput")
    bnsc_d = nc.dram_tensor("bnsc", [C], F32, kind="ExternalInput")
    bnbi_d = nc.dram_tensor("bnbi", [C], F32, kind="ExternalInput")
    out_d = nc.dram_tensor("out", [C, H, W], BF16, kind="ExternalOutput")

    taps = {}
    if debug_taps:
        for nm, shp in [("q", [CI, H, W]), ("k1", [CI, H, W]), ("v1", [CIN, H, W]),
                        ("expH", [96, W, 96]), ("A1", [CIN, H, W]), ("y", [C, H, W]),
                        ("k2", [CI, H, W])]:
            taps[nm] = nc.dram_tensor("t_" + nm, shp, F32, kind="ExternalOutput")

    ident = nc.inline_tensor(np.eye(128, dtype=ml_dtypes.bfloat16), "ident")
    ones_l = nc.inline_tensor(np.ones((96, 128), dtype=ml_dtypes.bfloat16), "ones_l")
    m01 = np.ones((96, 96), np.float32)
    np.fill_diagonal(m01, 0.0)
    m01_c = nc.inline_tensor(m01.astype(ml_dtypes.bfloat16), "m01c")
    epsv = nc.inline_tensor(np.full((128, 1), BN_EPS, np.float32), "epsv")

    st_d = nc.dram_tensor("st_i", [128, 8], F32)
    stg_d = nc.dram_tensor("stg_i", [128, 8], F32, addr_space="Shared")
    bar_d = nc.dram_tensor("bar_i", [128, 1], F32)
    barg_d = nc.dram_tensor("barg_i", [128, 1], F32, addr_space="Shared")

    NKV = CIN + C + CI  # 800: [vs 0:256 | xs 256:768 | ks 768:800]

    with tile.TileContext(nc) as tc, (
        tc.tile_pool(name="cst", bufs=1)) as cst, (
        tc.tile_pool(name="lng", bufs=1)) as lng, (
        tc.tile_pool(name="strm", bufs=2)) as strm, (
        tc.tile_pool(name="pe", bufs=2, space="PSUM")) as pe, (
        tc.tile_pool(name="ps", bufs=2, space="PSUM")) as ps, (
        tc.tile_pool(name="pa", bufs=1, space="PSUM")) as pa, (
        tc.tile_pool(name="pb", bufs=2, space="PSUM")) as pb:

        # ---------------- consts & weights ----------------
        id_t = cst.tile([128, 128], BF16, tag="id")
        nc.sync.dma_start(id_t[:], ident.ap()[:])
        ones_t = cst.tile([96, 128], BF16, tag="ones")
        nc.sync.dma_start(ones_t[:], ones_l.ap()[:])
        m01_t = cst.tile([96, 96], BF16, tag="m01")
        nc.sync.dma_start(m01_t[:], m01_c.ap()[:])
        eps_t = cst.tile([128, 1], F32, tag="eps")
        nc.sync.dma_start(eps_t[:], epsv.ap()[:])
        warm_t = cst.tile([128, 512], BF16, tag="warm")
        nc.vector.memset(warm_t[:], 0.0)

        def warm():
            # idle matmul on row-group 3 (partitions 96:128, untouched by the
            # real 96-partition operands) to keep the PE HAM duty monitor in
            # the warm (K=8/8, 2.4 GHz) state; runs concurrently, never read.
            dummy_t = pb.tile([128, 512], F32, tag="pmm", name="dummy")
            nc.tensor.matmul(dummy_t[0:32, :], id_t[96:128, 0:32], warm_t[96:128, :],
                             start=True, stop=True, tile_position=(96, 0))

        wq_t = [cst.tile([128, CI], BF16, tag=f"wq{k}", name=f"wq{k}") for k in range(4)]
        for k in range(4):
            nc.sync.dma_start(wq_t[k][:], wq_d.ap()[k * 128:(k + 1) * 128, :])
        wk2_t = [cst.tile([128, CI], BF16, tag=f"wk2{k}", name=f"wk2{k}") for k in range(2)]
        wv2_t = [[cst.tile([128, 128], BF16, tag=f"wv2{k}{m}", name=f"wv2{k}{m}") for m in range(2)] for k in range(2)]
        wfin_t = [[cst.tile([128, 128], BF16, tag=f"wf{k}{m}", name=f"wf{k}{m}") for m in range(4)] for k in range(2)]
        for k in range(2):
            nc.sync.dma_start(wk2_t[k][:], wk2_d.ap()[k * 128:(k + 1) * 128, :])
            for m in range(2):
                nc.sync.dma_start(wv2_t[k][m][:], wv2_d.ap()[k * 128:(k + 1) * 128, m * 128:(m + 1) * 128])
            for m in range(4):
                nc.sync.dma_start(wfin_t[k][m][:], wfin_d.ap()[k * 128:(k + 1) * 128, m * 128:(m + 1) * 128])
        bnsc_t = cst.tile([128, 4], F32, tag="bnsc")
        bnbi_t = cst.tile([128, 4], F32, tag="bnbi")
        nc.sync.dma_start(bnsc_t[:], bnsc_d.ap().rearrange("(m p) -> p m", p=128))
        nc.sync.dma_start(bnbi_t[:], bnbi_d.ap().rearrange("(m p) -> p m", p=128))

        # ---------------- long-lived small tensors (lng pool) ----------------
        xh = [lng.tile([128, H, HS], BF16, tag=f"xh{i}", name=f"xh{i}") for i in range(4)]
        s1p = lng.tile([128, 4, 24], F32, tag="s1p")
        s2p = lng.tile([128, 4, 24], F32, tag="s2p")
        st_t = lng.tile([128, 8], F32, tag="st")
        stg_t = lng.tile([128, 8], F32, tag="stg")
        mean_t = lng.tile([128, 4], F32, tag="mean")
        var_t = lng.tile([128, 4], F32, tag="var")
        m2_t = lng.tile([128, 4], F32, tag="m2")
        sd_t = lng.tile([128, 4], F32, tag="sd")
        ri_t = lng.tile([128, 4], F32, tag="ri")
        a_t = lng.tile([128, 4], F32, tag="abn")
        b_t = lng.tile([128, 4], F32, tag="bbn")

        # ---------------- mid pool: q/k/v (later reused as y storage) ---------
        mid_cm = tc.tile_pool(name="mid", bufs=1)
        mid = mid_cm.__enter__()
        qr_t = mid.tile([128, H, W], BF16, tag="qr_t")   # q' @0:32 and @64:96
        kk_t = mid.tile([128, H, W], BF16, tag="kk_t")   # k1' @0:32, k2' @64:96
        v1 = [mid.tile([128, H, W], BF16, tag=f"v1{i}", name=f"v1{i}") for i in range(2)]  # (c, h, w); becomes v2 in place
        ytile = [qr_t, kk_t, v1[0], v1[1]]  # y overlays these after attention

        # ---------------- Phase 1: high stream on small grid, upsample -------
        ld_cm = tc.tile_pool(name="ld", bufs=1)
        ld = ld_cm.__enter__()
        with tc.tile_pool(name="ph13", bufs=1) as ph:
            wkvx_t = [[ph.tile([128, 128], BF16, tag=f"wkvx{m}_{k}", name=f"wkvx{m}_{k}") for k in range(4)] for m in range(7)]
            for m in range(7):
                mw = min(128, NKV - m * 128)
                for k in range(4):
                    nc.sync.dma_start(wkvx_t[m][k][:, 0:mw],
                                      wkvx_d.ap()[k * 128:(k + 1) * 128, m * 128:m * 128 + mw])

            vs_t = [ph.tile([128, HS, HS], BF16, tag=f"vs{i}", name=f"vs{i}") for i in range(2)]
            xs_t = [ph.tile([128, HS, HS], BF16, tag=f"xs{i}", name=f"xs{i}") for i in range(4)]
            ks_t = ph.tile([CI, HS, HS], BF16, tag="ks")

            for n0 in range(0, PIXS, 512):
                nn = min(512, PIXS - n0)
                hi_c = [ld.tile([128, 512], BF16, tag=f"hic{k}", name=f"hic{k}", bufs=2) for k in range(4)]
                for k in range(4):
                    nc.gpsimd.dma_start(hi_c[k][:, 0:nn],
                                        high_d.ap().rearrange("c a b -> c (a b)")[k * 128:(k + 1) * 128, n0:n0 + nn])
                
                for m in range(7):
                    mw = min(128, NKV - m * 128)
                    pm = pb.tile([128, 512], F32, tag="pmm")
                    for k in range(4):
                        nc.tensor.matmul(pm[0:mw, 0:nn], wkvx_t[m][k][:, 0:mw],
                                         hi_c[k][:, 0:nn], start=(k == 0), stop=(k == 3))
                    if m < 2:
                        dst = vs_t[m][:].rearrange("c a b -> c (a b)")[:, n0:n0 + nn]
                    elif m < 6:
                        dst = xs_t[m - 2][:].rearrange("c a b -> c (a b)")[:, n0:n0 + nn]
                    else:
                        dst = ks_t[:].rearrange("c a b -> c (a b)")[:, n0:n0 + nn]
                    nc.scalar.activation(dst, pm[0:mw, 0:nn], AF.Copy)

            # upsample k1 (into kk[0:32]) and v1
            kw_t = ph.tile([CI, HS, W], BF16, tag="kw")
            _up_last(nc, nc.vector, kw_t[:], ks_t[:])
            _up_mid(nc, nc.vector, kk_t[0:32], kw_t[:])
            for ct in range(2):
                vw_t = ph.tile([128, HS, W], BF16, tag="vw", name="vw", bufs=2)
                _up_last(nc, nc.vector, vw_t[:], vs_t[ct][:])
                _up_mid(nc, nc.vector, v1[ct][:], vw_t[:])

            # xs -> xh (h-upsampled, h-major: (c, h96, w48)), stays in SBUF
            for ct in range(4):
                _up_mid(nc, nc.vector, xh[ct][:], xs_t[ct][:])

        # ---------------- Phase 2: q from low ----------------
        for n0 in range(0, PIX, 512):
            low_c = [ld.tile([128, 512], BF16, tag=f"hic{k}", name=f"lowc{k}", bufs=2) for k in range(4)]
            for k in range(4):
                nc.gpsimd.dma_start(low_c[k][:],
                                    low_d.ap().rearrange("c a b -> c (a b)")[k * 128:(k + 1) * 128, n0:n0 + 512])
            pm = pb.tile([CI, 512], F32, tag="pmm", name="pmq")
            for k in range(4):
                nc.tensor.matmul(pm[:], wq_t[k][:], low_c[k][:], start=(k == 0), stop=(k == 3))
            nc.scalar.activation(qr_t[0:32].rearrange("c a b -> c (a b)")[:, n0:n0 + 512], pm[:], AF.Copy)
        ld_cm.__exit__(None, None, None)

        # ---------------- attention scratch pools ----------------
        pA1_cm = tc.tile_pool(name="pA1", bufs=1)
        pA1 = pA1_cm.__enter__()
        A1 = [pA1.tile([128, H, W], BF16, tag=f"A1{i}", name=f"A1{i}") for i in range(2)]  # (c, h, w)
        exps_cm = tc.tile_pool(name="exps", bufs=1)
        exps = exps_cm.__enter__()
        expH = exps.tile([96, W, 96], BF16, tag="expH")   # [H', w, h]
        expW = exps.tile([96, H, 96], BF16, tag="expW")   # [W', h, w]
        sln_t = exps.tile([128, 24, 96], F32, tag="sln")  # ln(s) quarter-batch
        srf_t = exps.tile([128, 24, 96], BF16, tag="srf")  # 1/s quarter-batch

        # ---------------- attention helpers ----------------
        def energies():
            kq = kk_t[0:32]
            q = qr_t[0:32]
            mb = m01_t[:].unsqueeze(1).broadcast_to([96, 4, 96])
            for w0 in range(0, W, 4):
                if w0 % 8 == 0:
                    warm()
                pes = pe.tile([96, 4, 96], F32, tag="pe")
                for j in range(4):
                    w = w0 + j
                    nc.tensor.matmul(pes[:, j, :], kq[:, :, w], q[:, :, w], start=True, stop=True)
                nc.scalar.activation(expH[:, w0:w0 + 4, :], pes[:], AF.Exp)
                nc.vector.tensor_mul(expH[:, w0:w0 + 4, :], expH[:, w0:w0 + 4, :], mb)
            for h0 in range(0, H, 4):
                if h0 % 8 == 0:
                    warm()
                pes = pe.tile([96, 4, 96], F32, tag="pe")
                for j in range(4):
                    h = h0 + j
                    nc.tensor.matmul(pes[:, j, :], kq[:, h, :], q[:, h, :], start=True, stop=True)
                nc.scalar.activation(expW[:, h0:h0 + 4, :], pes[:], AF.Exp)

        def softmax_norm(sln, srf):
            # Batched: 12x Ln (one ACT table load) -> one Exp(-x) -> bf16 1/s tile,
            # then contiguous DVE mults for expH and strided DVE/GpSimd for expW.
            expWv = expW[:].rearrange("p h w -> p w h")
            for quarter in range(4):
                wbase = quarter * 24
                for w0 in range(wbase, wbase + 24, 4):
                    pss = ps.tile([128, 4, 96], F32, tag="ps")
                    nc.tensor.matmul(pss[:], ones_t[:], expH[:, w0:w0 + 4, :], start=True, stop=False)
                    nc.tensor.matmul(pss[:], ones_t[:], expWv[:, w0:w0 + 4, :], start=False, stop=True)
                    nc.scalar.activation(sln[:, w0 - wbase:w0 - wbase + 4, :], pss[:], AF.Ln)
                nc.scalar.activation(srf[:], sln[:], AF.Exp, scale=-1.0)
                for ci, w0 in enumerate(range(wbase, wbase + 24, 4)):
                    sr = srf[0:96, w0 - wbase:w0 - wbase + 4, :]
                    nc.vector.tensor_mul(expH[:, w0:w0 + 4, :], expH[:, w0:w0 + 4, :], sr)
                    eng = nc.gpsimd if ci % 2 == 0 else nc.vector
                    eng.tensor_mul(expWv[:, w0:w0 + 4, :], expWv[:, w0:w0 + 4, :], sr)

        def aggregate(rnd, v):
            # W direction first: per-row h, writes A1[c, h, w] contiguous
            for gi, h0 in enumerate(range(0, H, 4)):
                if gi % 2 == 0:
                    warm()
                ptg = pe.tile([96, 4, 256], BF16, tag="pe")
                for j in range(4):
                    h = h0 + j
                    for ct in range(2):
                        nc.tensor.transpose(ptg[:, j, ct * 128:(ct + 1) * 128], v[ct][:, h, :], id_t[:])
                vtc = strm.tile([96, 4, 256], BF16, tag="vtc")
                nc.scalar.activation(vtc[:], ptg[:], AF.Copy)
                for half in range(2):
                    pag = pa.tile([128, 4, 96], F32, tag=f"pa{half}")
                    for j in range(4):
                        nc.tensor.matmul(pag[:, j, :], vtc[:, j, half * 128:(half + 1) * 128],
                                         expW[:, h0 + j, :], start=True, stop=True)
                    if rnd == 0:
                        nc.vector.tensor_copy(A1[half][:, h0:h0 + 4, :], pag[:])
                    else:
                        nc.vector.scalar_tensor_tensor(A1[half][:, h0:h0 + 4, :], pag[:], 1.0,
                                                       A1[half][:, h0:h0 + 4, :], AL.mult, AL.add)
            # H direction: per-column w, strided accumulate into A1
            for gi, w0 in enumerate(range(0, W, 4)):
                if gi % 2 == 0:
                    warm()
                ptg = pe.tile([96, 4, 256], BF16, tag="pe")
                for j in range(4):
                    w = w0 + j
                    for ct in range(2):
                        nc.tensor.transpose(ptg[:, j, ct * 128:(ct + 1) * 128], v[ct][:, :, w], id_t[:])
                vtc = strm.tile([96, 4, 256], BF16, tag="vtc")
                nc.scalar.activation(vtc[:], ptg[:], AF.Copy)
                for half in range(2):
                    pag = pa.tile([128, 4, 96], F32, tag=f"pa{half}")
                    for j in range(4):
                        nc.tensor.matmul(pag[:, j, :], vtc[:, j, half * 128:(half + 1) * 128],
                                         expH[:, w0 + j, :], start=True, stop=True)
                    dst = A1[half][:].rearrange("c h w -> c w h")[:, w0:w0 + 4, :]
                    nc.vector.scalar_tensor_tensor(dst, pag[:], 1.0, dst, AL.mult, AL.add)

        # ---------------- round 1 ----------------
        energies()
        softmax_norm(sln_t, srf_t)
        if taps:
            nc.gpsimd.dma_start(taps["expH"].ap().rearrange("c a b -> c (a b)"),
                                expH[:].rearrange("c a b -> c (a b)"))
        aggregate(0, v1)

        # ---------------- round 2 prep (h-major A1 slices) ----------------
        if taps:
            nc.gpsimd.dma_start(taps["k1"].ap().rearrange("c a b -> c (a b)"),
                                kk_t[0:32].rearrange("c a b -> c (a b)"))
        for h0 in range(0, H, 4):
            pm = pb.tile([CI, 4, 96], F32, tag="pmm")
            nc.tensor.matmul(pm[:].rearrange("c a b -> c (a b)"), id_t[0:32, 0:CI],
                             kk_t[0:32][:, h0:h0 + 4, :].rearrange("c a b -> c (a b)"),
                             start=True, stop=False)
            for k in range(2):
                nc.tensor.matmul(pm[:].rearrange("c a b -> c (a b)"), wk2_t[k][:],
                                 A1[k][:].rearrange("c h w -> c (h w)")[:, h0 * 96:(h0 + 4) * 96],
                                 start=False, stop=(k == 1))
            nc.scalar.activation(kk_t[0:32][:, h0:h0 + 4, :], pm[:], AF.Copy)
        for h0 in range(0, H, 4):
            for m in range(2):
                pm = pb.tile([128, 4, 96], F32, tag="pmm")
                nc.tensor.matmul(pm[:].rearrange("c a b -> c (a b)"), id_t[:],
                                 v1[m][:, h0:h0 + 4, :].rearrange("c a b -> c (a b)"),
                                 start=True, stop=False)
                for k in range(2):
                    nc.tensor.matmul(pm[:].rearrange("c a b -> c (a b)"), wv2_t[k][m][:],
                                     A1[k][:].rearrange("c h w -> c (h w)")[:, h0 * 96:(h0 + 4) * 96],
                                     start=False, stop=(k == 1))
                nc.scalar.activation(v1[m][:, h0:h0 + 4, :], pm[:], AF.Copy)

        # ---------------- round 2 ----------------
        energies()
        if taps:
            nc.gpsimd.dma_start(taps["q"].ap().rearrange("c a b -> c (a b)"),
                                qr_t[0:32].rearrange("c a b -> c (a b)"))
            nc.gpsimd.dma_start(taps["k2"].ap().rearrange("c a b -> c (a b)"),
                                kk_t[0:32].rearrange("c a b -> c (a b)"))
        # q/k dead: pre-blend x1 (w-upsample of xh) into the future y tiles
        for m in range(2):
            _up_last(nc, nc.vector, ytile[m][:], xh[m][:])
        softmax_norm(sln_t, srf_t)
        aggregate(1, v1)
        # early inter-core barrier: absorbs cross-core skew here, where the
        # gpsimd queue is idle, so the BN-stats AllReduce later doesn't pay it
        nc.gpsimd.collective_compute("AllReduce", AL.add, ins=[bar_d.ap()], outs=[barg_d.ap()],
                                     replica_groups=[list(range(N_CORES))])

        # ---------------- debug taps -----------
        if taps:
            for ct in range(2):
                nc.gpsimd.dma_start(taps["v1"].ap().rearrange("c a b -> c (a b)")[ct * 128:(ct + 1) * 128, :],
                                    v1[ct][:].rearrange("c a b -> c (a b)"))
                nc.gpsimd.dma_start(taps["A1"].ap().rearrange("c a b -> c (a b)")[ct * 128:(ct + 1) * 128, :],
                                    A1[ct][:].rearrange("c a b -> c (a b)"))

        # v dead: pre-blend x1 for the remaining groups
        for m in range(2, 4):
            _up_last(nc, nc.vector, ytile[m][:], xh[m][:])

        # ---------------- attention scratch released; final pool -----------
        exps_cm.__exit__(None, None, None)
        fin_cm = tc.tile_pool(name="fin", bufs=1)
        fin = fin_cm.__enter__()

        # ---- final y (into SBUF, overlaying q/k/v) + per-group stats,
        # ---- AllReduce and BN-apply pipelined per 128-channel group m ----
        ndma = 0
        for m in range(4):
            for ci, h0 in enumerate(range(0, H, 4)):
                pm = pa.tile([128, 4, 96], F32, tag=f"pa{ci % 2}")
                for k in range(2):
                    nc.tensor.matmul(pm[:].rearrange("c a b -> c (a b)"), wfin_t[k][m][:],
                                     A1[k][:].rearrange("c h w -> c (h w)")[:, h0 * 96:(h0 + 4) * 96],
                                     start=(k == 0), stop=(k == 1))
                ydst = ytile[m][:, h0:h0 + 4, :]
                nc.vector.scalar_tensor_tensor(ydst, pm[:], 1.0, ydst, AL.mult, AL.add,
                                               accum_out=s1p[:, m, ci].unsqueeze(1))
                junk = fin.tile([128, 4, 96], BF16, tag="junk", bufs=2)
                nc.scalar.activation(junk[:], ydst, AF.Square, accum_out=s2p[:, m, ci].unsqueeze(1))

            # per-m partial BN stat reduction (overlaps remaining compute)
            nc.vector.tensor_reduce(st_t[:, 2 * m:2 * m + 1], s1p[:, m, :], mybir.AxisListType.X, AL.add)
            nc.vector.tensor_reduce(st_t[:, 2 * m + 1:2 * m + 2], s2p[:, m, :], mybir.AxisListType.X, AL.add)

        # ---------------- BN stats AllReduce (single collective) ----------
        nc.sync.dma_start(st_d.ap()[:], st_t[:])
        nc.gpsimd.collective_compute("AllReduce", AL.add, ins=[st_d.ap()], outs=[stg_d.ap()],
                                     replica_groups=[list(range(N_CORES))])
        nc.sync.dma_start(stg_t[:], stg_d.ap()[:])

        nc.vector.tensor_scalar_mul(mean_t[:], stg_t[:, 0:8:2], 1.0 / NTOT)
        nc.vector.tensor_scalar_mul(var_t[:], stg_t[:, 1:8:2], 1.0 / NTOT)
        nc.vector.tensor_mul(m2_t[:], mean_t[:], mean_t[:])
        nc.vector.tensor_sub(var_t[:], var_t[:], m2_t[:])
        nc.scalar.activation(sd_t[:], var_t[:], AF.Sqrt, bias=eps_t[:, 0:1])
        nc.vector.reciprocal(ri_t[:], sd_t[:])
        nc.vector.tensor_mul(a_t[:], ri_t[:], bnsc_t[:])
        nc.vector.tensor_mul(b_t[:], a_t[:], mean_t[:])
        nc.vector.tensor_sub(b_t[:], bnbi_t[:], b_t[:])

        # ---------------- BN apply + contiguous output DMAs ----------------
        for m in range(4):
            for ki, hb in enumerate(range(0, H, 24)):
                oc = fin.tile([128, 24, 96], BF16, tag="obn", bufs=4)
                ysl = ytile[m][:, hb:hb + 24, :]
                if ki < 2 and m >= 1:
                    nc.vector.tensor_scalar(oc[:], ysl, a_t[:, m:m + 1], b_t[:, m:m + 1],
                                            AL.mult, AL.add)
                    nc.vector.tensor_scalar_max(oc[:], oc[:], 0.0)
                else:
                    nc.scalar.activation(oc[:], ysl, AF.Relu,
                                         scale=a_t[:, m:m + 1], bias=b_t[:, m:m + 1])
                eng = nc.sync if (ndma % 2 == 0) else nc.gpsimd
                eng.dma_start(out_d.ap()[m * 128:(m + 1) * 128, hb:hb + 24, :], oc[:])
                ndma += 1

        if taps:
            for m in range(4):
                nc.gpsimd.dma_start(taps["y"].ap().rearrange("c a b -> c (a b)")[m * 128:(m + 1) * 128, :],
                                    ytile[m][:].rearrange("c a b -> c (a b)"))

        fin_cm.__exit__(None, None, None)
        pA1_cm.__exit__(None, None, None)
        mid_cm.__exit__(None, None, None)

    nc.compile()
    return nc


# ---------------------------------------------------------------------------
# host entry
# ---------------------------------------------------------------------------

def _host_prep(inputs):
    conv1_w = _f32(inputs["conv1_w"]); conv2_w = _f32(inputs["conv2_w"])
    q_w = _f32(inputs["q_w"]); k_w = _f32(inputs["k_w"]); v_w = _f32(inputs["v_w"])
    gamma = float(np.asarray(inputs["gamma"]))
    wb = _f32(inputs["bottleneck_w"])
    wb_v, wb_h = wb[:, :CIN], wb[:, CIN:]

    wq = (q_w @ conv1_w) / 16.0
    wvs = v_w @ conv2_w
    wxs = (wb_v @ conv2_w + wb_h) / 16.0
    wks = k_w @ conv2_w
    wkvx = np.concatenate([wvs, wxs, wks], axis=0)

    bf = ml_dtypes.bfloat16
    return {
        "wqT": np.ascontiguousarray(wq.T).astype(bf),
        "wkvxT": np.ascontiguousarray(wkvx.T).astype(bf),
        "wk2T": np.ascontiguousarray((gamma * k_w).T).astype(bf),
        "wv2T": np.ascontiguousarray((gamma * v_w).T).astype(bf),
        "wfinT": np.ascontiguousarray((gamma / 16.0 * wb_v).T).astype(bf),
        "bnsc": _f32(inputs["bn_scale"]),
        "bnbi": _f32(inputs["bn_bias"]),
    }


def _get_nc(debug_taps=False):
    key = ("nc", debug_taps)
    if key not in _CACHE:
        _CACHE[key] = build(debug_taps)
    return _CACHE[key]


def run(inputs, debug_taps=False, trace=False):
    for bname in ("conv1_b", "conv2_b", "q_b", "k_b", "v_b"):
        assert np.abs(np.asarray(inputs[bname])).max() == 0.0, f"nonzero {bname} unsupported"
    shared = _host_prep(inputs)
    low = _f32(inputs["low_feature"])
    high = _f32(inputs["high_feature"])
    in_maps = [dict(shared, low=low[i], high=high[i]) for i in range(N_CORES)]
    nc = _get_nc(debug_taps)
    res = run_bass_kernel_spmd(nc, in_maps, core_ids=list(range(N_CORES)), trace=trace)
    return res


def kernel(**inputs):
    res = run(inputs)
    out = np.stack([res.results[i]["out"] for i in range(N_CORES)], axis=0)
    return out.astype(np.float32)


# revision 40
# speedup vs baseline: 1.0330x; 1.0330x over previous
"""Trainium2 Bass kernel for nn_CCCrossLayerAttentionB (criss-cross cross-layer attention).

Self-contained: kernel(**inputs) -> np.ndarray [8, 512, 96, 96] fp32.

Sharding: data-parallel over batch (8 images -> 8 cores); BN stats via AllReduce.

Host-side restructuring (exact up to float assoc):
  - qf never materialized: q = (q_w @ conv1_w) @ low.
  - hf never materialized: every hf-consumer is a 1x1 conv, which commutes with the
    (separable, linear) bilinear 2x upsample, so the high stream is computed on the
    48x48 grid and upsampled afterward:
      ks = (k_w@conv2_w)@high; vs = (v_w@conv2_w)@high; xs = ((Wb_v@conv2_w+Wb_h)/16)@high
  - Device blends compute up16() = 16*up() in one fused op per output; the 1/16 is
    folded into host weights (q /16; xs /16; final att weights gamma/16).
  - vf2 never materialized:  k2' = (gamma k_w)@A1' + k1',  v2' = (gamma v_w)@A1' + v1',
    y = (gamma/16 Wb_v)@(A1'+A2') + up16(xs);  A' accumulated in place.
Attention per column w (and symmetrically per row h):
  e[H',h] = k'[:,H',w]^T q'[:,h,w] (flipped => softmax sums via ones-matmul, aggregation
  needs no attention transpose);  exp via ACT (+ -1e9 diag mask for the H direction);
  normalize exp tiles; aggregate with pixel-major v slices obtained from per-column
  PE transposes.
Everything stays in SBUF: A' is accumulated h-major so the final projection, BN stats
and BN apply all read/write contiguous APs; y reuses the attention tensors' SBUF and
only the fp32 output leaves the chip (large contiguous DMAs).
"""
import numpy as np
import ml_dtypes

import concourse.bass as bass
import concourse.bacc as bacc
import concourse.tile as tile
from concourse import mybir
from concourse.bass_utils import run_bass_kernel_spmd

F32 = mybir.dt.float32
BF16 = mybir.dt.bfloat16
AL = mybir.AluOpType
AF = mybir.ActivationFunctionType

N_CORES = 8
B, C, H, W = 8, 512, 96, 96
HS = 48
PIX = H * W
PIXS = HS * HS
CIN = 256
CI = 32
NTOT = float(B * PIX)
BN_EPS = 1e-5
NEG = -1e9

_CACHE = {}


def _f32(x):
    return np.ascontiguousarray(np.asarray(x, dtype=np.float32))


# ---------------------------------------------------------------------------
# blend helpers: up16 along last dim (48 -> 96) / middle dim
# ---------------------------------------------------------------------------

def _up_last(nc, eng, out, xin):
    """xin [P, n, 48] -> out [P, n, 96], out = 16 * bilinear (x4 per axis)."""
    eng.tensor_scalar_mul(out[:, :, 0:1], xin[:, :, 0:1], 4.0)
    eng.tensor_scalar_mul(out[:, :, 95:96], xin[:, :, 47:48], 4.0)
    eng.scalar_tensor_tensor(out[:, :, 2:95:2], xin[:, :, 1:48], 3.0, xin[:, :, 0:47],
                             AL.mult, AL.add)
    eng.scalar_tensor_tensor(out[:, :, 1:94:2], xin[:, :, 0:47], 3.0, xin[:, :, 1:48],
                             AL.mult, AL.add)


def _up_mid(nc, eng, out, xin):
    """xin [P, 48, n] -> out [P, 96, n]."""
    eng.tensor_scalar_mul(out[:, 0:1, :], xin[:, 0:1, :], 4.0)
    eng.tensor_scalar_mul(out[:, 95:96, :], xin[:, 47:48, :], 4.0)
    eng.scalar_tensor_tensor(out[:, 2:95:2, :], xin[:, 1:48, :], 3.0, xin[:, 0:47, :],
                             AL.mult, AL.add)
    eng.scalar_tensor_tensor(out[:, 1:94:2, :], xin[:, 0:47, :], 3.0, xin[:, 1:48, :],
                             AL.mult, AL.add)


def _up_mid_tt(nc, eng, out, xin, x3):
    """Same as _up_mid but pure tensor-tensor adds (GpSimd-safe); x3 = 3*xin."""
    eng.tensor_add(out[:, 0:1, :], x3[:, 0:1, :], xin[:, 0:1, :])
    eng.tensor_add(out[:, 95:96, :], x3[:, 47:48, :], xin[:, 47:48, :])
    eng.tensor_add(out[:, 2:95:2, :], x3[:, 1:48, :], xin[:, 0:47, :])
    eng.tensor_add(out[:, 1:94:2, :], x3[:, 0:47, :], xin[:, 1:48, :])


def _up_last_tt(nc, eng, out, xin, x3):
    """Same as _up_last but pure tensor-tensor adds (GpSimd-safe); x3 = 3*xin."""
    eng.tensor_add(out[:, :, 0:1], x3[:, :, 0:1], xin[:, :, 0:1])
    eng.tensor_add(out[:, :, 95:96], x3[:, :, 47:48], xin[:, :, 47:48])
    eng.tensor_add(out[:, :, 2:95:2], x3[:, :, 1:48], xin[:, :, 0:47])
    eng.tensor_add(out[:, :, 1:94:2], x3[:, :, 0:47], xin[:, :, 1:48])


# ---------------------------------------------------------------------------
# device kernel
# ---------------------------------------------------------------------------

def build(debug_taps=False):
    nc = bacc.Bacc("TRN2", target_bir_lowering=False, debug=False, num_devices=N_CORES)

    low_d = nc.dram_tensor("low", [C, H, W], F32, kind="ExternalInput")
    high_d = nc.dram_tensor("high", [C, HS, HS], F32, kind="ExternalInput")
    wq_d = nc.dram_tensor("wqT", [C, CI], BF16, kind="ExternalInput")
    wkvx_d = nc.dram_tensor("wkvxT", [C, CIN + C + CI], BF16, kind="ExternalInput")
    wk2_d = nc.dram_tensor("wk2T", [CIN, CI], BF16, kind="ExternalInput")
    wv2_d = nc.dram_tensor("wv2T", [CIN, CIN], BF16, kind="ExternalInput")
    wfin_d = nc.dram_tensor("wfinT", [CIN, C], BF16, kind="ExternalInput")
    bnsc_d = nc.dram_tensor("bnsc", [C], F32, kind="ExternalInput")
    bnbi_d = nc.dram_tensor("bnbi", [C], F32, kind="ExternalInput")
    out_d = nc.dram_tensor("out", [C, H, W], BF16, kind="ExternalOutput")

    taps = {}
    if debug_taps:
        for nm, shp in [("q", [CI, H, W]), ("k1", [CI, H, W]), ("v1", [CIN, H, W]),
                        ("expH", [96, W, 96]), ("A1", [CIN, H, W]), ("y", [C, H, W]),
                        ("k2", [CI, H, W])]:
            taps[nm] = nc.dram_tensor("t_" + nm, shp, F32, kind="ExternalOutput")

    ident = nc.inline_tensor(np.eye(128, dtype=ml_dtypes.bfloat16), "ident")
    ones_l = nc.inline_tensor(np.ones((96, 128), dtype=ml_dtypes.bfloat16), "ones_l")
    m01 = np.ones((96, 96), np.float32)
    np.fill_diagonal(m01, 0.0)
    m01_c = nc.inline_tensor(m01.astype(ml_dtypes.bfloat16), "m01c")
    epsv = nc.inline_tensor(np.full((128, 1), BN_EPS, np.float32), "epsv")

    st_d = nc.dram_tensor("st_i", [128, 8], F32)
    stg_d = nc.dram_tensor("stg_i", [128, 8], F32, addr_space="Shared")
    bar_d = nc.dram_tensor("bar_i", [128, 1], F32)
    barg_d = nc.dram_tensor("barg_i", [128, 1], F32, addr_space="Shared")

    NKV = CIN + C + CI  # 800: [vs 0:256 | xs 256:768 | ks 768:800]

    with tile.TileContext(nc) as tc, (
        tc.tile_pool(name="cst", bufs=1)) as cst, (
        tc.tile_pool(name="lng", bufs=1)) as lng, (
        tc.tile_pool(name="strm", bufs=2)) as strm, (
        tc.tile_pool(name="pe", bufs=2, space="PSUM")) as pe, (
        tc.tile_pool(name="ps", bufs=2, space="PSUM")) as ps, (
        tc.tile_pool(name="pa", bufs=1, space="PSUM")) as pa, (
        tc.tile_pool(name="pb", bufs=2, space="PSUM")) as pb:

        # ---------------- consts & weights ----------------
        id_t = cst.tile([128, 128], BF16, tag="id")
        nc.sync.dma_start(id_t[:], ident.ap()[:])
        ones_t = cst.tile([96, 128], BF16, tag="ones")
        nc.sync.dma_start(ones_t[:], ones_l.ap()[:])
        m01_t = cst.tile([96, 96], BF16, tag="m01")
        nc.sync.dma_start(m01_t[:], m01_c.ap()[:])
        eps_t = cst.tile([128, 1], F32, tag="eps")
        nc.sync.dma_start(eps_t[:], epsv.ap()[:])
        warm_t = cst.tile([128, 512], BF16, tag="warm")
        nc.vector.memset(warm_t[:], 0.0)

        def warm():
            # large idle matmul to keep the PE HAM duty monitor in the warm
            # (K=8/8, 2.4 GHz) state; output is never read.
            dummy_t = pb.tile([128, 512], F32, tag="pmm", name="dummy")
            nc.tensor.matmul(dummy_t[:], id_t[:], warm_t[:], start=True, stop=True)

        wq_t = [cst.tile([128, CI], BF16, tag=f"wq{k}", name=f"wq{k}") for k in range(4)]
        for k in range(4):
            nc.sync.dma_start(wq_t[k][:], wq_d.ap()[k * 128:(k + 1) * 128, :])
        wk2_t = [cst.tile([128, CI], BF16, tag=f"wk2{k}", name=f"wk2{k}") for k in range(2)]
        wv2_t = [[cst.tile([128, 128], BF16, tag=f"wv2{k}{m}", name=f"wv2{k}{m}") for m in range(2)] for k in range(2)]
        wfin_t = [[cst.tile([128, 128], BF16, tag=f"wf{k}{m}", name=f"wf{k}{m}") for m in range(4)] for k in range(2)]
        for k in range(2):
            nc.sync.dma_start(wk2_t[k][:], wk2_d.ap()[k * 128:(k + 1) * 128, :])
            for m in range(2):
                nc.sync.dma_start(wv2_t[k][m][:], wv2_d.ap()[k * 128:(k + 1) * 128, m * 128:(m + 1) * 128])
            for m in range(4):
                nc.sync.dma_start(wfin_t[k][m][:], wfin_d.ap()[k * 128:(k + 1) * 128, m * 128:(m + 1) * 128])
        bnsc_t = cst.tile([128, 4], F32, tag="bnsc")
        bnbi_t = cst.tile([128, 4], F32, tag="bnbi")
        nc.sync.dma_start(bnsc_t[:], bnsc_d.ap().rearrange("(m p) -> p m", p=128))
        nc.sync.dma_start(bnbi_t[:], bnbi_d.ap().rearrange("(m p) -> p m", p=128))

        # ---------------- long-lived small tensors (lng pool) ----------------
        xh = [lng.tile([128, H, HS], BF16, tag=f"xh{i}", name=f"xh{i}") for i in range(4)]
        s1p = lng.tile([128, 4, 24], F32, tag="s1p")
        s2p = lng.tile([128, 4, 24], F32, tag="s2p")
        st_t = lng.tile([128, 8], F32, tag="st")
        stg_t = lng.tile([128, 8], F32, tag="stg")
        mean_t = lng.tile([128, 4], F32, tag="mean")
        var_t = lng.tile([128, 4], F32, tag="var")
        m2_t = lng.tile([128, 4], F32, tag="m2")
        sd_t = lng.tile([128, 4], F32, tag="sd")
        ri_t = lng.tile([128, 4], F32, tag="ri")
        a_t = lng.tile([128, 4], F32, tag="abn")
        b_t = lng.tile([128, 4], F32, tag="bbn")

        # ---------------- mid pool: q/k/v (later reused as y storage) ---------
        mid_cm = tc.tile_pool(name="mid", bufs=1)
        mid = mid_cm.__enter__()
        qr_t = mid.tile([128, H, W], BF16, tag="qr_t")   # q' @0:32 and @64:96
        kk_t = mid.tile([128, H, W], BF16, tag="kk_t")   # k1' @0:32, k2' @64:96
        v1 = [mid.tile([128, H, W], BF16, tag=f"v1{i}", name=f"v1{i}") for i in range(2)]  # (c, h, w); becomes v2 in place
        ytile = [qr_t, kk_t, v1[0], v1[1]]  # y overlays these after attention

        # ---------------- Phase 1: high stream on small grid, upsample -------
        ld_cm = tc.tile_pool(name="ld", bufs=1)
        ld = ld_cm.__enter__()
        with tc.tile_pool(name="ph13", bufs=1) as ph:
            wkvx_t = [[ph.tile([128, 128], BF16, tag=f"wkvx{m}_{k}", name=f"wkvx{m}_{k}") for k in range(4)] for m in range(7)]
            for m in range(7):
                mw = min(128, NKV - m * 128)
                for k in range(4):
                    nc.sync.dma_start(wkvx_t[m][k][:, 0:mw],
                                      wkvx_d.ap()[k * 128:(k + 1) * 128, m * 128:m * 128 + mw])

            vs_t = [ph.tile([128, HS, HS], BF16, tag=f"vs{i}", name=f"vs{i}") for i in range(2)]
            xs_t = [ph.tile([128, HS, HS], BF16, tag=f"xs{i}", name=f"xs{i}") for i in range(4)]
            ks_t = ph.tile([CI, HS, HS], BF16, tag="ks")

            for n0 in range(0, PIXS, 512):
                nn = min(512, PIXS - n0)
                hi_c = [ld.tile([128, 512], BF16, tag=f"hic{k}", name=f"hic{k}", bufs=2) for k in range(4)]
                for k in range(4):
                    nc.gpsimd.dma_start(hi_c[k][:, 0:nn],
                                        high_d.ap().rearrange("c a b -> c (a b)")[k * 128:(k + 1) * 128, n0:n0 + nn])
                
                for m in range(7):
                    mw = min(128, NKV - m * 128)
                    pm = pb.tile([128, 512], F32, tag="pmm")
                    for k in range(4):
                        nc.tensor.matmul(pm[0:mw, 0:nn], wkvx_t[m][k][:, 0:mw],
                                         hi_c[k][:, 0:nn], start=(k == 0), stop=(k == 3))
                    if m < 2:
                        dst = vs_t[m][:].rearrange("c a b -> c (a b)")[:, n0:n0 + nn]
                    elif m < 6:
                        dst = xs_t[m - 2][:].rearrange("c a b -> c (a b)")[:, n0:n0 + nn]
                    else:
                        dst = ks_t[:].rearrange("c a b -> c (a b)")[:, n0:n0 + nn]
                    nc.scalar.activation(dst, pm[0:mw, 0:nn], AF.Copy)

            # upsample k1 (into kk[0:32]) and v1
            kw_t = ph.tile([CI, HS, W], BF16, tag="kw")
            _up_last(nc, nc.vector, kw_t[:], ks_t[:])
            _up_mid(nc, nc.vector, kk_t[0:32], kw_t[:])
            for ct in range(2):
                vw_t = ph.tile([128, HS, W], BF16, tag="vw", name="vw", bufs=2)
                _up_last(nc, nc.vector, vw_t[:], vs_t[ct][:])
                _up_mid(nc, nc.vector, v1[ct][:], vw_t[:])

            # xs -> xh (h-upsampled, h-major: (c, h96, w48)), stays in SBUF
            for ct in range(4):
                _up_mid(nc, nc.vector, xh[ct][:], xs_t[ct][:])

        # ---------------- Phase 2: q from low ----------------
        for n0 in range(0, PIX, 512):
            low_c = [ld.tile([128, 512], BF16, tag=f"hic{k}", name=f"lowc{k}", bufs=2) for k in range(4)]
            for k in range(4):
                nc.gpsimd.dma_start(low_c[k][:],
                                    low_d.ap().rearrange("c a b -> c (a b)")[k * 128:(k + 1) * 128, n0:n0 + 512])
            pm = pb.tile([CI, 512], F32, tag="pmm", name="pmq")
            for k in range(4):
                nc.tensor.matmul(pm[:], wq_t[k][:], low_c[k][:], start=(k == 0), stop=(k == 3))
            nc.scalar.activation(qr_t[0:32].rearrange("c a b -> c (a b)")[:, n0:n0 + 512], pm[:], AF.Copy)
        ld_cm.__exit__(None, None, None)

        # ---------------- attention scratch pools ----------------
        pA1_cm = tc.tile_pool(name="pA1", bufs=1)
        pA1 = pA1_cm.__enter__()
        A1 = [pA1.tile([128, H, W], BF16, tag=f"A1{i}", name=f"A1{i}") for i in range(2)]  # (c, h, w)
        exps_cm = tc.tile_pool(name="exps", bufs=1)
        exps = exps_cm.__enter__()
        expH = exps.tile([96, W, 96], BF16, tag="expH")   # [H', w, h]
        expW = exps.tile([96, H, 96], BF16, tag="expW")   # [W', h, w]
        sln_t = exps.tile([128, 24, 96], F32, tag="sln")  # ln(s) quarter-batch
        srf_t = exps.tile([128, 24, 96], BF16, tag="srf")  # 1/s quarter-batch

        # ---------------- attention helpers ----------------
        def energies():
            kq = kk_t[0:32]
            q = qr_t[0:32]
            mb = m01_t[:].unsqueeze(1).broadcast_to([96, 4, 96])
            for w0 in range(0, W, 4):
                if w0 % 16 == 0:
                    warm()
                pes = pe.tile([96, 4, 96], F32, tag="pe")
                for j in range(4):
                    w = w0 + j
                    nc.tensor.matmul(pes[:, j, :], kq[:, :, w], q[:, :, w], start=True, stop=True)
                nc.scalar.activation(expH[:, w0:w0 + 4, :], pes[:], AF.Exp)
                nc.vector.tensor_mul(expH[:, w0:w0 + 4, :], expH[:, w0:w0 + 4, :], mb)
            for h0 in range(0, H, 4):
                if h0 % 16 == 0:
                    warm()
                pes = pe.tile([96, 4, 96], F32, tag="pe")
                for j in range(4):
                    h = h0 + j
                    nc.tensor.matmul(pes[:, j, :], kq[:, h, :], q[:, h, :], start=True, stop=True)
                nc.scalar.activation(expW[:, h0:h0 + 4, :], pes[:], AF.Exp)

        def softmax_norm(sln, srf):
            # Batched: 12x Ln (one ACT table load) -> one Exp(-x) -> bf16 1/s tile,
            # then contiguous DVE mults for expH and strided DVE/GpSimd for expW.
            expWv = expW[:].rearrange("p h w -> p w h")
            for quarter in range(4):
                wbase = quarter * 24
                for w0 in range(wbase, wbase + 24, 4):
                    pss = ps.tile([128, 4, 96], F32, tag="ps")
                    nc.tensor.matmul(pss[:], ones_t[:], expH[:, w0:w0 + 4, :], start=True, stop=False)
                    nc.tensor.matmul(pss[:], ones_t[:], expWv[:, w0:w0 + 4, :], start=False, stop=True)
                    nc.scalar.activation(sln[:, w0 - wbase:w0 - wbase + 4, :], pss[:], AF.Ln)
                nc.scalar.activation(srf[:], sln[:], AF.Exp, scale=-1.0)
                for ci, w0 in enumerate(range(wbase, wbase + 24, 4)):
                    sr = srf[0:96, w0 - wbase:w0 - wbase + 4, :]
                    nc.vector.tensor_mul(expH[:, w0:w0 + 4, :], expH[:, w0:w0 + 4, :], sr)
                    eng = nc.gpsimd if ci % 2 == 0 else nc.vector
                    eng.tensor_mul(expWv[:, w0:w0 + 4, :], expWv[:, w0:w0 + 4, :], sr)

        def aggregate(rnd, v):
            # W direction first: per-row h, writes A1[c, h, w] contiguous
            for gi, h0 in enumerate(range(0, H, 4)):
                if gi % 3 == 0:
                    warm()
                ptg = pe.tile([96, 4, 256], BF16, tag="pe")
                for j in range(4):
                    h = h0 + j
                    for ct in range(2):
                        nc.tensor.transpose(ptg[:, j, ct * 128:(ct + 1) * 128], v[ct][:, h, :], id_t[:])
                vtc = strm.tile([96, 4, 256], BF16, tag="vtc")
                nc.scalar.activation(vtc[:], ptg[:], AF.Copy)
                for half in range(2):
                    pag = pa.tile([128, 4, 96], F32, tag=f"pa{half}")
                    for j in range(4):
                        nc.tensor.matmul(pag[:, j, :], vtc[:, j, half * 128:(half + 1) * 128],
                                         expW[:, h0 + j, :], start=True, stop=True)
                    if rnd == 0:
                        nc.vector.tensor_copy(A1[half][:, h0:h0 + 4, :], pag[:])
                    else:
                        nc.vector.scalar_tensor_tensor(A1[half][:, h0:h0 + 4, :], pag[:], 1.0,
                                                       A1[half][:, h0:h0 + 4, :], AL.mult, AL.add)
            # H direction: per-column w, strided accumulate into A1
            for gi, w0 in enumerate(range(0, W, 4)):
                if gi % 3 == 0:
                    warm()
                ptg = pe.tile([96, 4, 256], BF16, tag="pe")
                for j in range(4):
                    w = w0 + j
                    for ct in range(2):
                        nc.tensor.transpose(ptg[:, j, ct * 128:(ct + 1) * 128], v[ct][:, :, w], id_t[:])
                vtc = strm.tile([96, 4, 256], BF16, tag="vtc")
                nc.scalar.activation(vtc[:], ptg[:], AF.Copy)
                for half in range(2):
                    pag = pa.tile([128, 4, 96], F32, tag=f"pa{half}")
                    for j in range(4):
                        nc.tensor.matmul(pag[:, j, :], vtc[:, j, half * 128:(half + 1) * 128],
                                         expH[:, w0 + j, :], start=True, stop=True)
                    dst = A1[half][:].rearrange("c h w -> c w h")[:, w0:w0 + 4, :]
                    nc.vector.scalar_tensor_tensor(dst, pag[:], 1.0, dst, AL.mult, AL.add)

        # ---------------- round 1 ----------------
        energies()
        softmax_norm(sln_t, srf_t)
        if taps:
            nc.gpsimd.dma_start(taps["expH"].ap().rearrange("c a b -> c (a b)"),
                                expH[:].rearrange("c a b -> c (a b)"))
        aggregate(0, v1)

        # ---------------- round 2 prep (h-major A1 slices) ----------------
        if taps:
            nc.gpsimd.dma_start(taps["k1"].ap().rearrange("c a b -> c (a b)"),
                                kk_t[0:32].rearrange("c a b -> c (a b)"))
        for h0 in range(0, H, 4):
            pm = pb.tile([CI, 4, 96], F32, tag="pmm")
            nc.tensor.matmul(pm[:].rearrange("c a b -> c (a b)"), id_t[0:32, 0:CI],
                             kk_t[0:32][:, h0:h0 + 4, :].rearrange("c a b -> c (a b)"),
                             start=True, stop=False)
            for k in range(2):
                nc.tensor.matmul(pm[:].rearrange("c a b -> c (a b)"), wk2_t[k][:],
                                 A1[k][:].rearrange("c h w -> c (h w)")[:, h0 * 96:(h0 + 4) * 96],
                                 start=False, stop=(k == 1))
            nc.scalar.activation(kk_t[0:32][:, h0:h0 + 4, :], pm[:], AF.Copy)
        for h0 in range(0, H, 4):
            for m in range(2):
                pm = pb.tile([128, 4, 96], F32, tag="pmm")
                nc.tensor.matmul(pm[:].rearrange("c a b -> c (a b)"), id_t[:],
                                 v1[m][:, h0:h0 + 4, :].rearrange("c a b -> c (a b)"),
                                 start=True, stop=False)
                for k in range(2):
                    nc.tensor.matmul(pm[:].rearrange("c a b -> c (a b)"), wv2_t[k][m][:],
                                     A1[k][:].rearrange("c h w -> c (h w)")[:, h0 * 96:(h0 + 4) * 96],
                                     start=False, stop=(k == 1))
                nc.scalar.activation(v1[m][:, h0:h0 + 4, :], pm[:], AF.Copy)

        # ---------------- round 2 ----------------
        energies()
        if taps:
            nc.gpsimd.dma_start(taps["q"].ap().rearrange("c a b -> c (a b)"),
                                qr_t[0:32].rearrange("c a b -> c (a b)"))
            nc.gpsimd.dma_start(taps["k2"].ap().rearrange("c a b -> c (a b)"),
                                kk_t[0:32].rearrange("c a b -> c (a b)"))
        # q/k dead: pre-blend x1 (w-upsample of xh) into the future y tiles
        for m in range(2):
            _up_last(nc, nc.vector, ytile[m][:], xh[m][:])
        softmax_norm(sln_t, srf_t)
        aggregate(1, v1)
        # early inter-core barrier: absorbs cross-core skew here, where the
        # gpsimd queue is idle, so the BN-stats AllReduce later doesn't pay it
        nc.gpsimd.collective_compute("AllReduce", AL.add, ins=[bar_d.ap()], outs=[barg_d.ap()],
                                     replica_groups=[list(range(N_CORES))])

        # ---------------- debug taps -----------
        if taps:
            for ct in range(2):
                nc.gpsimd.dma_start(taps["v1"].ap().rearrange("c a b -> c (a b)")[ct * 128:(ct + 1) * 128, :],
                                    v1[ct][:].rearrange("c a b -> c (a b)"))
                nc.gpsimd.dma_start(taps["A1"].ap().rearrange("c a b -> c (a b)")[ct * 128:(ct + 1) * 128, :],
                                    A1[ct][:].rearrange("c a b -> c (a b)"))

        # v dead: pre-blend x1 for the remaining groups
        for m in range(2, 4):
            _up_last(nc, nc.vector, ytile[m][:], xh[m][:])

        # ---------------- attention scratch released; final pool -----------
        exps_cm.__exit__(None, None, None)
        fin_cm = tc.tile_pool(name="fin", bufs=1)
        fin = fin_cm.__enter__()

        # ---- final y (into SBUF, overlaying q/k/v) + per-group stats,
        # ---- AllReduce and BN-apply pipelined per 128-channel group m ----
        ndma = 0
        for m in range(4):
            for ci, h0 in enumerate(range(0, H, 4)):
                pm = pa.tile([128, 4, 96], F32, tag=f"pa{ci % 2}")
                for k in range(2):
                    nc.tensor.matmul(pm[:].rearrange("c a b -> c (a b)"), wfin_t[k][m][:],
                                     A1[k][:].rearrange("c h w -> c (h w)")[:, h0 * 96:(h0 + 4) * 96],
                                     start=(k == 0), stop=(k == 1))
                ydst = ytile[m][:, h0:h0 + 4, :]
                nc.vector.scalar_tensor_tensor(ydst, pm[:], 1.0, ydst, AL.mult, AL.add,
                                               accum_out=s1p[:, m, ci].unsqueeze(1))
                junk = fin.tile([128, 4, 96], BF16, tag="junk", bufs=2)
                nc.scalar.activation(junk[:], ydst, AF.Square, accum_out=s2p[:, m, ci].unsqueeze(1))

            # per-m partial BN stat reduction (overlaps remaining compute)
            nc.vector.tensor_reduce(st_t[:, 2 * m:2 * m + 1], s1p[:, m, :], mybir.AxisListType.X, AL.add)
            nc.vector.tensor_reduce(st_t[:, 2 * m + 1:2 * m + 2], s2p[:, m, :], mybir.AxisListType.X, AL.add)

        # ---------------- BN stats AllReduce (single collective) ----------
        nc.sync.dma_start(st_d.ap()[:], st_t[:])
        nc.gpsimd.collective_compute("AllReduce", AL.add, ins=[st_d.ap()], outs=[stg_d.ap()],
                                     replica_groups=[list(range(N_CORES))])
        nc.sync.dma_start(stg_t[:], stg_d.ap()[:])

        nc.vector.tensor_scalar_mul(mean_t[:], stg_t[:, 0:8:2], 1.0 / NTOT)
        nc.vector.tensor_scalar_mul(var_t[:], stg_t[:, 1:8:2], 1.0 / NTOT)
        nc.vector.tensor_mul(m2_t[:], mean_t[:], mean_t[:])
        nc.vector.tensor_sub(var_t[:], var_t[:], m2_t[:])
        nc.scalar.activation(sd_t[:], var_t[:], AF.Sqrt, bias=eps_t[:, 0:1])
        nc.vector.reciprocal(ri_t[:], sd_t[:])
        nc.vector.tensor_mul(a_t[:], ri_t[:], bnsc_t[:])
        nc.vector.tensor_mul(b_t[:], a_t[:], mean_t[:])
        nc.vector.tensor_sub(b_t[:], bnbi_t[:], b_t[:])

        # ---------------- BN apply + contiguous output DMAs ----------------
        for m in range(4):
            for ki, hb in enumerate(range(0, H, 24)):
                oc = fin.tile([128, 24, 96], BF16, tag="obn", bufs=4)
                ysl = ytile[m][:, hb:hb + 24, :]
                if ki < 2 and m >= 1:
                    nc.vector.tensor_scalar(oc[:], ysl, a_t[:, m:m + 1], b_t[:, m:m + 1],
                                            AL.mult, AL.add)
                    nc.vector.tensor_scalar_max(oc[:], oc[:], 0.0)
                else:
                    nc.scalar.activation(oc[:], ysl, AF.Relu,
                                         scale=a_t[:, m:m + 1], bias=b_t[:, m:m + 1])
                eng = nc.sync if (ndma % 2 == 0) else nc.gpsimd
                eng.dma_start(out_d.ap()[m * 128:(m + 1) * 128, hb:hb + 24, :], oc[:])
                ndma += 1

        if taps:
            for m in range(4):
                nc.gpsimd.dma_start(taps["y"].ap().rearrange("c a b -> c (a b)")[m * 128:(m + 1) * 128, :],
                                    ytile[m][:].rearrange("c a b -> c (a b)"))

        fin_cm.__exit__(None, None, None)
        pA1_cm.__exit__(None, None, None)
        mid_cm.__exit__(None, None, None)

    nc.compile()
    return nc


# ---------------------------------------------------------------------------
# host entry
# ---------------------------------------------------------------------------

def _host_prep(inputs):
    conv1_w = _f32(inputs["conv1_w"]); conv2_w = _f32(inputs["conv2_w"])
    q_w = _f32(inputs["q_w"]); k_w = _f32(inputs["k_w"]); v_w = _f32(inputs["v_w"])
    gamma = float(np.asarray(inputs["gamma"]))
    wb = _f32(inputs["bottleneck_w"])
    wb_v, wb_h = wb[:, :CIN], wb[:, CIN:]

    wq = (q_w @ conv1_w) / 16.0
    wvs = v_w @ conv2_w
    wxs = (wb_v @ conv2_w + wb_h) / 16.0
    wks = k_w @ conv2_w
    wkvx = np.concatenate([wvs, wxs, wks], axis=0)

    bf = ml_dtypes.bfloat16
    return {
        "wqT": np.ascontiguousarray(wq.T).astype(bf),
        "wkvxT": np.ascontiguousarray(wkvx.T).astype(bf),
        "wk2T": np.ascontiguousarray((gamma * k_w).T).astype(bf),
        "wv2T": np.ascontiguousarray((gamma * v_w).T).astype(bf),
        "wfinT": np.ascontiguousarray((gamma / 16.0 * wb_v).T).astype(bf),
        "bnsc": _f32(inputs["bn_scale"]),
        "bnbi": _f32(inputs["bn_bias"]),
    }


def _get_nc(debug_taps=False):
    key = ("nc", debug_taps)
    if key not in _CACHE:
        _CACHE[key] = build(debug_taps)
    return _CACHE[key]


def run(inputs, debug_taps=False, trace=False):
    for bname in ("conv1_b", "conv2_b", "q_b", "k_b", "v_b"):
        assert np.abs(np.asarray(inputs[bname])).max() == 0.0, f"nonzero {bname} unsupported"
    shared = _host_prep(inputs)
    low = _f32(inputs["low_feature"])
    high = _f32(inputs["high_feature"])
    in_maps = [dict(shared, low=low[i], high=high[i]) for i in range(N_CORES)]
    nc = _get_nc(debug_taps)
    res = run_bass_kernel_spmd(nc, in_maps, core_ids=list(range(N_CORES)), trace=trace)
    return res


def kernel(**inputs):
    res = run(inputs)
    out = np.stack([res.results[i]["out"] for i in range(N_CORES)], axis=0)
    return out.astype(np.float32)


# revision 41
# speedup vs baseline: 1.0794x; 1.0449x over previous
"""Trainium2 Bass kernel for nn_CCCrossLayerAttentionB (criss-cross cross-layer attention).

Self-contained: kernel(**inputs) -> np.ndarray [8, 512, 96, 96] fp32.

Sharding: data-parallel over batch (8 images -> 8 cores); BN stats via AllReduce.

Host-side restructuring (exact up to float assoc):
  - qf never materialized: q = (q_w @ conv1_w) @ low.
  - hf never materialized: every hf-consumer is a 1x1 conv, which commutes with the
    (separable, linear) bilinear 2x upsample, so the high stream is computed on the
    48x48 grid and upsampled afterward:
      ks = (k_w@conv2_w)@high; vs = (v_w@conv2_w)@high; xs = ((Wb_v@conv2_w+Wb_h)/16)@high
  - Device blends compute up16() = 16*up() in one fused op per output; the 1/16 is
    folded into host weights (q /16; xs /16; final att weights gamma/16).
  - vf2 never materialized:  k2' = (gamma k_w)@A1' + k1',  v2' = (gamma v_w)@A1' + v1',
    y = (gamma/16 Wb_v)@(A1'+A2') + up16(xs);  A' accumulated in place.
Attention per column w (and symmetrically per row h):
  e[H',h] = k'[:,H',w]^T q'[:,h,w] (flipped => softmax sums via ones-matmul, aggregation
  needs no attention transpose);  exp via ACT (+ -1e9 diag mask for the H direction);
  normalize exp tiles; aggregate with pixel-major v slices obtained from per-column
  PE transposes.
Everything stays in SBUF: A' is accumulated h-major so the final projection, BN stats
and BN apply all read/write contiguous APs; y reuses the attention tensors' SBUF and
only the fp32 output leaves the chip (large contiguous DMAs).
"""
import numpy as np
import ml_dtypes

import concourse.bass as bass
import concourse.bacc as bacc
import concourse.tile as tile
from concourse import mybir
from concourse.bass_utils import run_bass_kernel_spmd

F32 = mybir.dt.float32
BF16 = mybir.dt.bfloat16
AL = mybir.AluOpType
AF = mybir.ActivationFunctionType

N_CORES = 8
B, C, H, W = 8, 512, 96, 96
HS = 48
PIX = H * W
PIXS = HS * HS
CIN = 256
CI = 32
NTOT = float(B * PIX)
BN_EPS = 1e-5
NEG = -1e9

_CACHE = {}


def _f32(x):
    return np.ascontiguousarray(np.asarray(x, dtype=np.float32))


# ---------------------------------------------------------------------------
# blend helpers: up16 along last dim (48 -> 96) / middle dim
# ---------------------------------------------------------------------------

def _up_last(nc, eng, out, xin):
    """xin [P, n, 48] -> out [P, n, 96], out = 16 * bilinear (x4 per axis)."""
    eng.tensor_scalar_mul(out[:, :, 0:1], xin[:, :, 0:1], 4.0)
    eng.tensor_scalar_mul(out[:, :, 95:96], xin[:, :, 47:48], 4.0)
    eng.scalar_tensor_tensor(out[:, :, 2:95:2], xin[:, :, 1:48], 3.0, xin[:, :, 0:47],
                             AL.mult, AL.add)
    eng.scalar_tensor_tensor(out[:, :, 1:94:2], xin[:, :, 0:47], 3.0, xin[:, :, 1:48],
                             AL.mult, AL.add)


def _up_mid(nc, eng, out, xin):
    """xin [P, 48, n] -> out [P, 96, n]."""
    eng.tensor_scalar_mul(out[:, 0:1, :], xin[:, 0:1, :], 4.0)
    eng.tensor_scalar_mul(out[:, 95:96, :], xin[:, 47:48, :], 4.0)
    eng.scalar_tensor_tensor(out[:, 2:95:2, :], xin[:, 1:48, :], 3.0, xin[:, 0:47, :],
                             AL.mult, AL.add)
    eng.scalar_tensor_tensor(out[:, 1:94:2, :], xin[:, 0:47, :], 3.0, xin[:, 1:48, :],
                             AL.mult, AL.add)


def _up_mid_tt(nc, eng, out, xin, x3):
    """Same as _up_mid but pure tensor-tensor adds (GpSimd-safe); x3 = 3*xin."""
    eng.tensor_add(out[:, 0:1, :], x3[:, 0:1, :], xin[:, 0:1, :])
    eng.tensor_add(out[:, 95:96, :], x3[:, 47:48, :], xin[:, 47:48, :])
    eng.tensor_add(out[:, 2:95:2, :], x3[:, 1:48, :], xin[:, 0:47, :])
    eng.tensor_add(out[:, 1:94:2, :], x3[:, 0:47, :], xin[:, 1:48, :])


def _up_last_tt(nc, eng, out, xin, x3):
    """Same as _up_last but pure tensor-tensor adds (GpSimd-safe); x3 = 3*xin."""
    eng.tensor_add(out[:, :, 0:1], x3[:, :, 0:1], xin[:, :, 0:1])
    eng.tensor_add(out[:, :, 95:96], x3[:, :, 47:48], xin[:, :, 47:48])
    eng.tensor_add(out[:, :, 2:95:2], x3[:, :, 1:48], xin[:, :, 0:47])
    eng.tensor_add(out[:, :, 1:94:2], x3[:, :, 0:47], xin[:, :, 1:48])


# ---------------------------------------------------------------------------
# device kernel
# ---------------------------------------------------------------------------

def build(debug_taps=False):
    nc = bacc.Bacc("TRN2", target_bir_lowering=False, debug=False, num_devices=N_CORES)

    low_d = nc.dram_tensor("low", [C, H, W], F32, kind="ExternalInput")
    high_d = nc.dram_tensor("high", [C, HS, HS], F32, kind="ExternalInput")
    wq_d = nc.dram_tensor("wqT", [C, CI], BF16, kind="ExternalInput")
    wkvx_d = nc.dram_tensor("wkvxT", [C, CIN + C + CI], BF16, kind="ExternalInput")
    wk2_d = nc.dram_tensor("wk2T", [CIN, CI], BF16, kind="ExternalInput")
    wv2_d = nc.dram_tensor("wv2T", [CIN, CIN], BF16, kind="ExternalInput")
    wfin_d = nc.dram_tensor("wfinT", [CIN, C], BF16, kind="ExternalInput")
    bnsc_d = nc.dram_tensor("bnsc", [C], F32, kind="ExternalInput")
    bnbi_d = nc.dram_tensor("bnbi", [C], F32, kind="ExternalInput")
    out_d = nc.dram_tensor("out", [C, H, W], BF16, kind="ExternalOutput")

    taps = {}
    if debug_taps:
        for nm, shp in [("q", [CI, H, W]), ("k1", [CI, H, W]), ("v1", [CIN, H, W]),
                        ("expH", [96, W, 96]), ("A1", [CIN, H, W]), ("y", [C, H, W]),
                        ("k2", [CI, H, W])]:
            taps[nm] = nc.dram_tensor("t_" + nm, shp, F32, kind="ExternalOutput")

    ident = nc.inline_tensor(np.eye(128, dtype=ml_dtypes.bfloat16), "ident")
    ones_l = nc.inline_tensor(np.ones((96, 128), dtype=ml_dtypes.bfloat16), "ones_l")
    m01 = np.ones((96, 96), np.float32)
    np.fill_diagonal(m01, 0.0)
    m01_c = nc.inline_tensor(m01.astype(ml_dtypes.bfloat16), "m01c")
    epsv = nc.inline_tensor(np.full((128, 1), BN_EPS, np.float32), "epsv")

    st_d = nc.dram_tensor("st_i", [128, 8], F32)
    stg_d = nc.dram_tensor("stg_i", [128, 8], F32, addr_space="Shared")
    bar_d = nc.dram_tensor("bar_i", [128, 1], F32)
    barg_d = nc.dram_tensor("barg_i", [128, 1], F32, addr_space="Shared")

    NKV = CIN + C + CI  # 800: [vs 0:256 | xs 256:768 | ks 768:800]

    with tile.TileContext(nc) as tc, (
        tc.tile_pool(name="cst", bufs=1)) as cst, (
        tc.tile_pool(name="lng", bufs=1)) as lng, (
        tc.tile_pool(name="strm", bufs=2)) as strm, (
        tc.tile_pool(name="pe", bufs=2, space="PSUM")) as pe, (
        tc.tile_pool(name="ps", bufs=2, space="PSUM")) as ps, (
        tc.tile_pool(name="pa", bufs=1, space="PSUM")) as pa, (
        tc.tile_pool(name="pb", bufs=2, space="PSUM")) as pb:

        # ---------------- consts & weights ----------------
        id_t = cst.tile([128, 128], BF16, tag="id")
        nc.sync.dma_start(id_t[:], ident.ap()[:])
        ones_t = cst.tile([96, 128], BF16, tag="ones")
        nc.sync.dma_start(ones_t[:], ones_l.ap()[:])
        m01_t = cst.tile([96, 96], BF16, tag="m01")
        nc.sync.dma_start(m01_t[:], m01_c.ap()[:])
        eps_t = cst.tile([128, 1], F32, tag="eps")
        nc.sync.dma_start(eps_t[:], epsv.ap()[:])
        warm_t = cst.tile([128, 512], BF16, tag="warm")
        nc.vector.memset(warm_t[:], 0.0)

        def warm():
            # large idle matmul to keep the PE HAM duty monitor in the warm
            # (K=8/8, 2.4 GHz) state; output is never read.
            dummy_t = pb.tile([128, 512], F32, tag="pmm", name="dummy")
            nc.tensor.matmul(dummy_t[:], id_t[:], warm_t[:], start=True, stop=True)

        wq_t = [cst.tile([128, CI], BF16, tag=f"wq{k}", name=f"wq{k}") for k in range(4)]
        for k in range(4):
            nc.sync.dma_start(wq_t[k][:], wq_d.ap()[k * 128:(k + 1) * 128, :])
        wk2_t = [cst.tile([128, CI], BF16, tag=f"wk2{k}", name=f"wk2{k}") for k in range(2)]
        wv2_t = [[cst.tile([128, 128], BF16, tag=f"wv2{k}{m}", name=f"wv2{k}{m}") for m in range(2)] for k in range(2)]
        wfin_t = [[cst.tile([128, 128], BF16, tag=f"wf{k}{m}", name=f"wf{k}{m}") for m in range(4)] for k in range(2)]
        for k in range(2):
            nc.sync.dma_start(wk2_t[k][:], wk2_d.ap()[k * 128:(k + 1) * 128, :])
            for m in range(2):
                nc.sync.dma_start(wv2_t[k][m][:], wv2_d.ap()[k * 128:(k + 1) * 128, m * 128:(m + 1) * 128])
            for m in range(4):
                nc.sync.dma_start(wfin_t[k][m][:], wfin_d.ap()[k * 128:(k + 1) * 128, m * 128:(m + 1) * 128])
        bnsc_t = cst.tile([128, 4], F32, tag="bnsc")
        bnbi_t = cst.tile([128, 4], F32, tag="bnbi")
        nc.sync.dma_start(bnsc_t[:], bnsc_d.ap().rearrange("(m p) -> p m", p=128))
        nc.sync.dma_start(bnbi_t[:], bnbi_d.ap().rearrange("(m p) -> p m", p=128))

        # ---------------- long-lived small tensors (lng pool) ----------------
        xh = [lng.tile([128, H, HS], BF16, tag=f"xh{i}", name=f"xh{i}") for i in range(4)]
        s1p = lng.tile([128, 4, 24], F32, tag="s1p")
        s2p = lng.tile([128, 4, 24], F32, tag="s2p")
        st_t = lng.tile([128, 8], F32, tag="st")
        stg_t = lng.tile([128, 8], F32, tag="stg")
        mean_t = lng.tile([128, 4], F32, tag="mean")
        var_t = lng.tile([128, 4], F32, tag="var")
        m2_t = lng.tile([128, 4], F32, tag="m2")
        sd_t = lng.tile([128, 4], F32, tag="sd")
        ri_t = lng.tile([128, 4], F32, tag="ri")
        a_t = lng.tile([128, 4], F32, tag="abn")
        b_t = lng.tile([128, 4], F32, tag="bbn")

        # ---------------- mid pool: q/k/v (later reused as y storage) ---------
        mid_cm = tc.tile_pool(name="mid", bufs=1)
        mid = mid_cm.__enter__()
        qr_t = mid.tile([128, H, W], BF16, tag="qr_t")   # q' @0:32 and @64:96
        kk_t = mid.tile([128, H, W], BF16, tag="kk_t")   # k1' @0:32, k2' @64:96
        v1 = [mid.tile([128, H, W], BF16, tag=f"v1{i}", name=f"v1{i}") for i in range(2)]  # (c, h, w); becomes v2 in place
        ytile = [qr_t, kk_t, v1[0], v1[1]]  # y overlays these after attention

        # ---------------- Phase 1: high stream on small grid, upsample -------
        ld_cm = tc.tile_pool(name="ld", bufs=1)
        ld = ld_cm.__enter__()
        with tc.tile_pool(name="ph13", bufs=1) as ph:
            wkvx_t = [[ph.tile([128, 128], BF16, tag=f"wkvx{m}_{k}", name=f"wkvx{m}_{k}") for k in range(4)] for m in range(7)]
            for m in range(7):
                mw = min(128, NKV - m * 128)
                for k in range(4):
                    nc.sync.dma_start(wkvx_t[m][k][:, 0:mw],
                                      wkvx_d.ap()[k * 128:(k + 1) * 128, m * 128:m * 128 + mw])

            vs_t = [ph.tile([128, HS, HS], BF16, tag=f"vs{i}", name=f"vs{i}") for i in range(2)]
            xs_t = [ph.tile([128, HS, HS], BF16, tag=f"xs{i}", name=f"xs{i}") for i in range(4)]
            ks_t = ph.tile([CI, HS, HS], BF16, tag="ks")

            for n0 in range(0, PIXS, 512):
                nn = min(512, PIXS - n0)
                hi_c = [ld.tile([128, 512], BF16, tag=f"hic{k}", name=f"hic{k}", bufs=2) for k in range(4)]
                for k in range(4):
                    nc.gpsimd.dma_start(hi_c[k][:, 0:nn],
                                        high_d.ap().rearrange("c a b -> c (a b)")[k * 128:(k + 1) * 128, n0:n0 + nn])
                
                for m in range(7):
                    mw = min(128, NKV - m * 128)
                    pm = pb.tile([128, 512], F32, tag="pmm")
                    for k in range(4):
                        nc.tensor.matmul(pm[0:mw, 0:nn], wkvx_t[m][k][:, 0:mw],
                                         hi_c[k][:, 0:nn], start=(k == 0), stop=(k == 3))
                    if m < 2:
                        dst = vs_t[m][:].rearrange("c a b -> c (a b)")[:, n0:n0 + nn]
                    elif m < 6:
                        dst = xs_t[m - 2][:].rearrange("c a b -> c (a b)")[:, n0:n0 + nn]
                    else:
                        dst = ks_t[:].rearrange("c a b -> c (a b)")[:, n0:n0 + nn]
                    nc.scalar.activation(dst, pm[0:mw, 0:nn], AF.Copy)

            # upsample k1 (into kk[0:32]) and v1
            kw_t = ph.tile([CI, HS, W], BF16, tag="kw")
            _up_last(nc, nc.vector, kw_t[:], ks_t[:])
            _up_mid(nc, nc.vector, kk_t[0:32], kw_t[:])
            for ct in range(2):
                vw_t = ph.tile([128, HS, W], BF16, tag="vw", name="vw", bufs=2)
                _up_last(nc, nc.vector, vw_t[:], vs_t[ct][:])
                _up_mid(nc, nc.vector, v1[ct][:], vw_t[:])

            # xs -> xh (h-upsampled, h-major: (c, h96, w48)), stays in SBUF
            for ct in range(4):
                _up_mid(nc, nc.vector, xh[ct][:], xs_t[ct][:])

        # ---------------- Phase 2: q from low ----------------
        for n0 in range(0, PIX, 512):
            low_c = [ld.tile([128, 512], BF16, tag=f"hic{k}", name=f"lowc{k}", bufs=2) for k in range(4)]
            for k in range(4):
                nc.gpsimd.dma_start(low_c[k][:],
                                    low_d.ap().rearrange("c a b -> c (a b)")[k * 128:(k + 1) * 128, n0:n0 + 512])
            pm = pb.tile([CI, 512], F32, tag="pmm", name="pmq")
            for k in range(4):
                nc.tensor.matmul(pm[:], wq_t[k][:], low_c[k][:], start=(k == 0), stop=(k == 3))
            nc.scalar.activation(qr_t[0:32].rearrange("c a b -> c (a b)")[:, n0:n0 + 512], pm[:], AF.Copy)
        ld_cm.__exit__(None, None, None)

        # ---------------- attention scratch pools ----------------
        pA1_cm = tc.tile_pool(name="pA1", bufs=1)
        pA1 = pA1_cm.__enter__()
        A1 = [pA1.tile([128, H, W], BF16, tag=f"A1{i}", name=f"A1{i}") for i in range(2)]  # (c, h, w)
        exps_cm = tc.tile_pool(name="exps", bufs=1)
        exps = exps_cm.__enter__()
        expH = exps.tile([96, W, 96], BF16, tag="expH")   # [H', w, h]
        expW = exps.tile([96, H, 96], BF16, tag="expW")   # [W', h, w]
        sln_t = exps.tile([128, 24, 96], F32, tag="sln")  # ln(s) quarter-batch
        srf_t = exps.tile([128, 24, 96], BF16, tag="srf")  # 1/s quarter-batch

        # ---------------- attention helpers ----------------
        def energies():
            kq = kk_t[0:32]
            q = qr_t[0:32]
            mb = m01_t[:].unsqueeze(1).broadcast_to([96, 4, 96])
            for w0 in range(0, W, 4):
                if w0 % 16 == 0:
                    warm()
                pes = pe.tile([96, 4, 96], F32, tag="pe")
                for j in range(4):
                    w = w0 + j
                    nc.tensor.matmul(pes[:, j, :], kq[:, :, w], q[:, :, w], start=True, stop=True)
                nc.scalar.activation(expH[:, w0:w0 + 4, :], pes[:], AF.Exp)
                nc.vector.tensor_mul(expH[:, w0:w0 + 4, :], expH[:, w0:w0 + 4, :], mb)
            for h0 in range(0, H, 4):
                if h0 % 16 == 0:
                    warm()
                pes = pe.tile([96, 4, 96], F32, tag="pe")
                for j in range(4):
                    h = h0 + j
                    nc.tensor.matmul(pes[:, j, :], kq[:, h, :], q[:, h, :], start=True, stop=True)
                nc.scalar.activation(expW[:, h0:h0 + 4, :], pes[:], AF.Exp)

        def softmax_agg(rnd, v, sln, srf):
            # Fused: per quarter, normalize then immediately run the H-direction
            # aggregation for those 24 columns (its deps are quarter-local), so
            # PE transposes/matmuls overlap the next quarter's normalization
            # instead of the two phases serializing. The W direction (which
            # needs fully-normalized expW) follows and always accumulates.
            expWv = expW[:].rearrange("p h w -> p w h")
            for quarter in range(4):
                wbase = quarter * 24
                for w0 in range(wbase, wbase + 24, 4):
                    pss = ps.tile([128, 4, 96], F32, tag="ps")
                    nc.tensor.matmul(pss[:], ones_t[:], expH[:, w0:w0 + 4, :], start=True, stop=False)
                    nc.tensor.matmul(pss[:], ones_t[:], expWv[:, w0:w0 + 4, :], start=False, stop=True)
                    nc.scalar.activation(sln[:, w0 - wbase:w0 - wbase + 4, :], pss[:], AF.Ln)
                nc.scalar.activation(srf[:], sln[:], AF.Exp, scale=-1.0)
                for ci, w0 in enumerate(range(wbase, wbase + 24, 4)):
                    sr = srf[0:96, w0 - wbase:w0 - wbase + 4, :]
                    nc.vector.tensor_mul(expH[:, w0:w0 + 4, :], expH[:, w0:w0 + 4, :], sr)
                    eng = nc.gpsimd if ci % 2 == 0 else nc.vector
                    eng.tensor_mul(expWv[:, w0:w0 + 4, :], expWv[:, w0:w0 + 4, :], sr)
                for gi, w0 in enumerate(range(wbase, wbase + 24, 4)):
                    if gi % 3 == 0:
                        warm()
                    ptg = pe.tile([96, 4, 256], BF16, tag="pe")
                    for j in range(4):
                        w = w0 + j
                        for ct in range(2):
                            nc.tensor.transpose(ptg[:, j, ct * 128:(ct + 1) * 128], v[ct][:, :, w], id_t[:])
                    vtc = strm.tile([96, 4, 256], BF16, tag="vtc")
                    nc.scalar.activation(vtc[:], ptg[:], AF.Copy)
                    for half in range(2):
                        pag = pa.tile([128, 4, 96], F32, tag=f"pa{half}")
                        for j in range(4):
                            nc.tensor.matmul(pag[:, j, :], vtc[:, j, half * 128:(half + 1) * 128],
                                             expH[:, w0 + j, :], start=True, stop=True)
                        dst = A1[half][:].rearrange("c h w -> c w h")[:, w0:w0 + 4, :]
                        if rnd == 0:
                            nc.vector.tensor_copy(dst, pag[:])
                        else:
                            nc.vector.scalar_tensor_tensor(dst, pag[:], 1.0, dst, AL.mult, AL.add)
            # W direction: per-row h, contiguous accumulate into A1
            for gi, h0 in enumerate(range(0, H, 4)):
                if gi % 3 == 0:
                    warm()
                ptg = pe.tile([96, 4, 256], BF16, tag="pe")
                for j in range(4):
                    h = h0 + j
                    for ct in range(2):
                        nc.tensor.transpose(ptg[:, j, ct * 128:(ct + 1) * 128], v[ct][:, h, :], id_t[:])
                vtc = strm.tile([96, 4, 256], BF16, tag="vtc")
                nc.scalar.activation(vtc[:], ptg[:], AF.Copy)
                for half in range(2):
                    pag = pa.tile([128, 4, 96], F32, tag=f"pa{half}")
                    for j in range(4):
                        nc.tensor.matmul(pag[:, j, :], vtc[:, j, half * 128:(half + 1) * 128],
                                         expW[:, h0 + j, :], start=True, stop=True)
                    nc.vector.scalar_tensor_tensor(A1[half][:, h0:h0 + 4, :], pag[:], 1.0,
                                                   A1[half][:, h0:h0 + 4, :], AL.mult, AL.add)

        # ---------------- round 1 ----------------
        energies()
        softmax_agg(0, v1, sln_t, srf_t)
        if taps:
            nc.gpsimd.dma_start(taps["expH"].ap().rearrange("c a b -> c (a b)"),
                                expH[:].rearrange("c a b -> c (a b)"))

        # ---------------- round 2 prep (h-major A1 slices) ----------------
        if taps:
            nc.gpsimd.dma_start(taps["k1"].ap().rearrange("c a b -> c (a b)"),
                                kk_t[0:32].rearrange("c a b -> c (a b)"))
        for h0 in range(0, H, 4):
            pm = pb.tile([CI, 4, 96], F32, tag="pmm")
            nc.tensor.matmul(pm[:].rearrange("c a b -> c (a b)"), id_t[0:32, 0:CI],
                             kk_t[0:32][:, h0:h0 + 4, :].rearrange("c a b -> c (a b)"),
                             start=True, stop=False)
            for k in range(2):
                nc.tensor.matmul(pm[:].rearrange("c a b -> c (a b)"), wk2_t[k][:],
                                 A1[k][:].rearrange("c h w -> c (h w)")[:, h0 * 96:(h0 + 4) * 96],
                                 start=False, stop=(k == 1))
            nc.scalar.activation(kk_t[0:32][:, h0:h0 + 4, :], pm[:], AF.Copy)
        for h0 in range(0, H, 4):
            for m in range(2):
                pm = pb.tile([128, 4, 96], F32, tag="pmm")
                nc.tensor.matmul(pm[:].rearrange("c a b -> c (a b)"), id_t[:],
                                 v1[m][:, h0:h0 + 4, :].rearrange("c a b -> c (a b)"),
                                 start=True, stop=False)
                for k in range(2):
                    nc.tensor.matmul(pm[:].rearrange("c a b -> c (a b)"), wv2_t[k][m][:],
                                     A1[k][:].rearrange("c h w -> c (h w)")[:, h0 * 96:(h0 + 4) * 96],
                                     start=False, stop=(k == 1))
                nc.scalar.activation(v1[m][:, h0:h0 + 4, :], pm[:], AF.Copy)

        # ---------------- round 2 ----------------
        energies()
        if taps:
            nc.gpsimd.dma_start(taps["q"].ap().rearrange("c a b -> c (a b)"),
                                qr_t[0:32].rearrange("c a b -> c (a b)"))
            nc.gpsimd.dma_start(taps["k2"].ap().rearrange("c a b -> c (a b)"),
                                kk_t[0:32].rearrange("c a b -> c (a b)"))
        # q/k dead: pre-blend x1 (w-upsample of xh) into the future y tiles
        for m in range(2):
            _up_last(nc, nc.vector, ytile[m][:], xh[m][:])
        softmax_agg(1, v1, sln_t, srf_t)
        # early inter-core barrier: absorbs cross-core skew here, where the
        # gpsimd queue is idle, so the BN-stats AllReduce later doesn't pay it
        nc.gpsimd.collective_compute("AllReduce", AL.add, ins=[bar_d.ap()], outs=[barg_d.ap()],
                                     replica_groups=[list(range(N_CORES))])

        # ---------------- debug taps -----------
        if taps:
            for ct in range(2):
                nc.gpsimd.dma_start(taps["v1"].ap().rearrange("c a b -> c (a b)")[ct * 128:(ct + 1) * 128, :],
                                    v1[ct][:].rearrange("c a b -> c (a b)"))
                nc.gpsimd.dma_start(taps["A1"].ap().rearrange("c a b -> c (a b)")[ct * 128:(ct + 1) * 128, :],
                                    A1[ct][:].rearrange("c a b -> c (a b)"))

        # v dead: pre-blend x1 for the remaining groups
        for m in range(2, 4):
            _up_last(nc, nc.vector, ytile[m][:], xh[m][:])

        # ---------------- attention scratch released; final pool -----------
        exps_cm.__exit__(None, None, None)
        fin_cm = tc.tile_pool(name="fin", bufs=1)
        fin = fin_cm.__enter__()

        # ---- final y (into SBUF, overlaying q/k/v) + per-group stats,
        # ---- AllReduce and BN-apply pipelined per 128-channel group m ----
        ndma = 0
        for m in range(4):
            for ci, h0 in enumerate(range(0, H, 4)):
                pm = pa.tile([128, 4, 96], F32, tag=f"pa{ci % 2}")
                for k in range(2):
                    nc.tensor.matmul(pm[:].rearrange("c a b -> c (a b)"), wfin_t[k][m][:],
                                     A1[k][:].rearrange("c h w -> c (h w)")[:, h0 * 96:(h0 + 4) * 96],
                                     start=(k == 0), stop=(k == 1))
                ydst = ytile[m][:, h0:h0 + 4, :]
                nc.vector.scalar_tensor_tensor(ydst, pm[:], 1.0, ydst, AL.mult, AL.add,
                                               accum_out=s1p[:, m, ci].unsqueeze(1))
                junk = fin.tile([128, 4, 96], BF16, tag="junk", bufs=2)
                nc.scalar.activation(junk[:], ydst, AF.Square, accum_out=s2p[:, m, ci].unsqueeze(1))

            # per-m partial BN stat reduction (overlaps remaining compute)
            nc.vector.tensor_reduce(st_t[:, 2 * m:2 * m + 1], s1p[:, m, :], mybir.AxisListType.X, AL.add)
            nc.vector.tensor_reduce(st_t[:, 2 * m + 1:2 * m + 2], s2p[:, m, :], mybir.AxisListType.X, AL.add)

        # ---------------- BN stats AllReduce (single collective) ----------
        nc.sync.dma_start(st_d.ap()[:], st_t[:])
        nc.gpsimd.collective_compute("AllReduce", AL.add, ins=[st_d.ap()], outs=[stg_d.ap()],
                                     replica_groups=[list(range(N_CORES))])
        nc.sync.dma_start(stg_t[:], stg_d.ap()[:])

        nc.vector.tensor_scalar_mul(mean_t[:], stg_t[:, 0:8:2], 1.0 / NTOT)
        nc.vector.tensor_scalar_mul(var_t[:], stg_t[:, 1:8:2], 1.0 / NTOT)
        nc.vector.tensor_mul(m2_t[:], mean_t[:], mean_t[:])
        nc.vector.tensor_sub(var_t[:], var_t[:], m2_t[:])
        nc.scalar.activation(sd_t[:], var_t[:], AF.Sqrt, bias=eps_t[:, 0:1])
        nc.vector.reciprocal(ri_t[:], sd_t[:])
        nc.vector.tensor_mul(a_t[:], ri_t[:], bnsc_t[:])
        nc.vector.tensor_mul(b_t[:], a_t[:], mean_t[:])
        nc.vector.tensor_sub(b_t[:], bnbi_t[:], b_t[:])

        # ---------------- BN apply + contiguous output DMAs ----------------
        for m in range(4):
            for ki, hb in enumerate(range(0, H, 24)):
                oc = fin.tile([128, 24, 96], BF16, tag="obn", bufs=4)
                ysl = ytile[m][:, hb:hb + 24, :]
                if ki < 2 and m >= 1:
                    nc.vector.tensor_scalar(oc[:], ysl, a_t[:, m:m + 1], b_t[:, m:m + 1],
                                            AL.mult, AL.add)
                    nc.vector.tensor_scalar_max(oc[:], oc[:], 0.0)
                else:
                    nc.scalar.activation(oc[:], ysl, AF.Relu,
                                         scale=a_t[:, m:m + 1], bias=b_t[:, m:m + 1])
                eng = nc.sync if (ndma % 2 == 0) else nc.gpsimd
                eng.dma_start(out_d.ap()[m * 128:(m + 1) * 128, hb:hb + 24, :], oc[:])
                ndma += 1

        if taps:
            for m in range(4):
                nc.gpsimd.dma_start(taps["y"].ap().rearrange("c a b -> c (a b)")[m * 128:(m + 1) * 128, :],
                                    ytile[m][:].rearrange("c a b -> c (a b)"))

        fin_cm.__exit__(None, None, None)
        pA1_cm.__exit__(None, None, None)
        mid_cm.__exit__(None, None, None)

    nc.compile()
    return nc


# ---------------------------------------------------------------------------
# host entry
# ---------------------------------------------------------------------------

def _host_prep(inputs):
    conv1_w = _f32(inputs["conv1_w"]); conv2_w = _f32(inputs["conv2_w"])
    q_w = _f32(inputs["q_w"]); k_w = _f32(inputs["k_w"]); v_w = _f32(inputs["v_w"])
    gamma = float(np.asarray(inputs["gamma"]))
    wb = _f32(inputs["bottleneck_w"])
    wb_v, wb_h = wb[:, :CIN], wb[:, CIN:]

    wq = (q_w @ conv1_w) / 16.0
    wvs = v_w @ conv2_w
    wxs = (wb_v @ conv2_w + wb_h) / 16.0
    wks = k_w @ conv2_w
    wkvx = np.concatenate([wvs, wxs, wks], axis=0)

    bf = ml_dtypes.bfloat16
    return {
        "wqT": np.ascontiguousarray(wq.T).astype(bf),
        "wkvxT": np.ascontiguousarray(wkvx.T).astype(bf),
        "wk2T": np.ascontiguousarray((gamma * k_w).T).astype(bf),
        "wv2T": np.ascontiguousarray((gamma * v_w).T).astype(bf),
        "wfinT": np.ascontiguousarray((gamma / 16.0 * wb_v).T).astype(bf),
        "bnsc": _f32(inputs["bn_scale"]),
        "bnbi": _f32(inputs["bn_bias"]),
    }


def _get_nc(debug_taps=False):
    key = ("nc", debug_taps)
    if key not in _CACHE:
        _CACHE[key] = build(debug_taps)
    return _CACHE[key]


def run(inputs, debug_taps=False, trace=False):
    for bname in ("conv1_b", "conv2_b", "q_b", "k_b", "v_b"):
        assert np.abs(np.asarray(inputs[bname])).max() == 0.0, f"nonzero {bname} unsupported"
    shared = _host_prep(inputs)
    low = _f32(inputs["low_feature"])
    high = _f32(inputs["high_feature"])
    in_maps = [dict(shared, low=low[i], high=high[i]) for i in range(N_CORES)]
    nc = _get_nc(debug_taps)
    res = run_bass_kernel_spmd(nc, in_maps, core_ids=list(range(N_CORES)), trace=trace)
    return res


def kernel(**inputs):
    res = run(inputs)
    out = np.stack([res.results[i]["out"] for i in range(N_CORES)], axis=0)
    return out.astype(np.float32)
